# revision 20
# baseline (speedup 1.0000x reference)
"""Trainium2 Bass kernel for CausalAttentionSortNet bucket-scoring.

Math (see reference): only `k` feeds the output. For each merged batch*head
slice, the cumulative-average of k is sampled at bucket starts (every 128th
row), which reduces to per-chunk sums + a strictly-triangular prefix matmul.
The rest is tiny per-bucket sort projections and a 64x65 masked softmax.

Sharding: data-parallel over the merged (batch*heads)=32 axis across 8 cores,
4 slices per core as 2 pairs; partition=(slice_in_pair, chunk), free=(row, dim)
so every partition's k data is one contiguous 32KB HBM run. Both pairs of each
row-group share one SBUF tile so each fold is a single batched instruction.

`q` (half of all input bytes) is never read by the reference computation, so
it is not even transferred to the device.

Per row-group sub-tile: contiguous pairwise fold chains on DVE (unit-stride
fp32 tensor_tensor), with GpSimd taking a d-column share of the first level
in parallel. The folded row 0 feeds the PE prefix matmul (PT). F (row 0 of
chunk) is read straight out of sub-tile 0 before the folds clobber it - no
separate gather DMA. The 64x65 softmax keeps the zero-logit column explicit
in PSUM so the tail is one exp + one scale-mask per pair. Junk matmuls gated
on the stream keep the PE's HAM clock at 8/8 for the tail matmuls.
"""

from contextlib import ExitStack

import numpy as np

import concourse.bacc as bacc
import concourse.mybir as mybir
import concourse.tile as tile
from concourse import bass_utils

# Problem constants (hardcoded per contract; kernel.py must be self-contained).
B, HEADS, BUCKETS, DIM, DIM_SORT, T = 4, 8, 64, 64, 8, 8192
BH = B * HEADS            # 32 merged batch*head slices
NCORES = 8
BHC = BH // NCORES        # 4 slices per core
NPAIR = BHC // 2          # 2 pairs per core
CHUNK = T // BUCKETS      # 128 rows per bucket
NEG = -1.0e30             # softmax mask value (underflows exp to exactly 0)
FP = mybir.dt.float32
BF = mybir.dt.bfloat16

# rows-per-sub-tile (per pair); ascending-then-descending so folds start early
# and the tail tile is tiny. Sum = 128. All folds run on DVE: GpSimd
# tensor_tensor is ~3x slower AND degrades concurrent DVE ops ~4x (measured),
# so handing it any fold share is a net loss.
ROWS = (12, 24, 32, 32, 16, 8, 4)

TRACE = False  # set by test.py for profiling runs
TRACE_KWARGS = {}  # extra run_bass_kernel_spmd kwargs for profiling runs
LAST_RESULTS = None  # BassKernelResults of the most recent run

_PROG_CACHE = {}


def _chain(rs):
    """Pairwise fold schedule for rs rows.

    Returns (ops, final) where ops are in-place (dst_lo, dst_hi, src_lo,
    src_hi) folds and final = (row_a, row_b) whose sum is the column total
    (written to the separate contiguous par tile so it can be a matmul lhsT).
    """
    ops = []
    leftovers = []
    n = rs
    while n > 3:
        h = n // 2
        ops.append((0, h, h, 2 * h))
        if n % 2:
            leftovers.append(n - 1)
        n = h
    if n == 3:
        ops.append((1, 2, 2, 3))
        n = 2
    assert n == 2, rs
    for r in leftovers:
        ops.append((1, 2, r, r + 1))
    return ops, (0, 1)


def _build_program(enable_asserts=False):
    assert sum(ROWS) == CHUNK, (ROWS, CHUNK)
    nsub = len(ROWS)

    nc = bacc.Bacc(
        "TRN2",
        target_bir_lowering=False,
        debug=False,
        enable_asserts=enable_asserts,
        num_devices=NCORES,
    )

    def din(name, shape, dt=FP):
        return nc.dram_tensor(name, shape, dt, kind="ExternalInput").ap()

    kin = din("kin", (BHC, T, DIM))
    # all fp32 constants packed into ONE tensor / ONE DMA (the tile scheduler
    # has only 8 DMA completion-sem lanes; every extra DMA makes a later
    # bulk-DMA issue wait on an in-flight predecessor):
    # cols 0:514    = c128 block [lmat_s | idents | ident | amask65 | mmask65]
    # cols 514:930  = c64 block  [wqk_pt_p0 | wqk_pt_p1 | wqk_ft_p0 | wqk_ft_p1]
    # cols 930:1186 = c104 block (rows 0:104; per pair cq/ck const terms)
    cM = din("cM", (128, 1186))
    # cb: bf16 identity for PE warm-up matmuls
    cb = din("cb", (128, 128), BF)
    # out layout (b, i, pair, col): 520B contiguous per (b, i) partition
    rout = nc.dram_tensor(
        "rout", (2, BUCKETS, NPAIR, BUCKETS + 1), FP, kind="ExternalOutput"
    ).ap()

    Exp = mybir.ActivationFunctionType.Exp
    MULT = mybir.AluOpType.mult
    X = mybir.AxisListType.X

    with tile.TileContext(nc) as tc:
        with ExitStack() as ctx:
            singles = ctx.enter_context(tc.tile_pool(name="singles", bufs=1))
            kpool = ctx.enter_context(tc.tile_pool(name="kpool", bufs=1))
            small = ctx.enter_context(tc.tile_pool(name="small", bufs=2))
            pp = ctx.enter_context(tc.tile_pool(name="pp", bufs=1, space="PSUM"))

            # ---- everything streams on the one sync queue, in priority order:
            # sub-tile 0 (split into half-row DMAs to deepen the early SDMA
            # queue during the bandwidth ramp), then the constants, then the
            # remaining sub-tiles. Pairs share one tile so folds batch both.
            ksrcs = [
                kin[2 * p : 2 * p + 2].rearrange("b (c r) d -> (b c) r d", r=CHUNK)
                for p in range(NPAIR)
            ]
            kts = []
            r0 = 0
            for s, rs in enumerate(ROWS):
                kt = kpool.tile([128, NPAIR, rs, DIM], FP, tag=f"kt{s}")
                for p in range(NPAIR):
                    nc.sync.dma_start(kt[:, p], ksrcs[p][:, r0 : r0 + rs, :])
                kts.append(kt)
                r0 += rs
                if s == 0:
                    # constants ride the same queue right behind sub-tile 0
                    cb_sb = singles.tile([128, 128], BF, tag="cb")
                    nc.sync.dma_start(cb_sb[:], cb)
                    cM_sb = singles.tile([128, 1186], FP, tag="cM")
                    nc.sync.dma_start(cM_sb[:], cM)

            lmat_s = cM_sb[:, 0:128]
            idents = cM_sb[:, 128:256]
            ident = cM_sb[:, 256:384]
            amask = cM_sb[:, 384 : 384 + 65]
            mmask = cM_sb[:, 449 : 449 + 65]
            c64_sb = cM_sb[:, 514:930]
            c104_sb = cM_sb[0:104, 930:1186]

            # ---- PSUM tiles
            PT_ps = pp.tile([128, 128], FP, tag="PT")
            FT_ps = pp.tile([128, 128], FP, tag="FT")
            SKQs = [
                pp.tile([104, 128], FP, tag=f"SKQ{p}", name=f"SKQ{p}")
                for p in range(NPAIR)
            ]
            R_ps = pp.tile([128, NPAIR, BUCKETS + 1], FP, tag="R")
            dummy = pp.tile([128, 128], FP, tag="dummy")

            # ---- early PE work (consts-gated): R mask seeds + SKQ const seeds
            # one start=True per PSUM bank: start clears has_written bank-wide,
            # so only the first seed may carry it
            for p in range(NPAIR):
                nc.tensor.matmul(
                    R_ps[:, p, :], lhsT=ident, rhs=amask,
                    start=p == 0, stop=False, skip_group_check=True,
                )
            for p in range(NPAIR):
                nc.tensor.matmul(
                    SKQs[p][:], lhsT=ident[0:104, 0:104],
                    rhs=c104_sb[:, 128 * p : 128 * p + 128],
                    start=True, stop=False, skip_group_check=True,
                )

            # ---- F path: copy row 0 of sub-tile 0 to a contiguous tile
            # (matmul lhsT needs one flat free dim) before folds clobber it
            kt0 = kts[0]
            F_sb = small.tile([128, NPAIR, DIM], FP, tag="F")
            nc.vector.tensor_copy(F_sb[:], kt0[:, :, 0, :])
            nc.tensor.matmul(
                PT_ps[:], lhsT=F_sb[:], rhs=idents,
                start=True, stop=False, skip_group_check=True,
            )
            nc.tensor.matmul(
                FT_ps[:], lhsT=F_sb[:], rhs=ident, start=True, stop=True
            )
            FT_sb = small.tile([128, 128], FP, tag="FTs")
            nc.scalar.copy(FT_sb[:], FT_ps[:])
            for p in range(NPAIR):
                prow = slice(64 * p, 64 * p + 64)
                nc.tensor.matmul(
                    SKQs[p][:],
                    lhsT=c64_sb[prow, 208 + 104 * p : 208 + 104 * p + 104],
                    rhs=FT_sb[prow, :],
                    start=False, stop=False, skip_group_check=True,
                )

            # ---- per-sub-tile: batched fold chain then PT accumulation.
            # Power-of-2 tiles stop folding at 4 rows: the last DVE level
            # writes a row-major par2 tile (contiguous (pair, d) rows) and the
            # PE absorbs the final add as two accumulating matmuls.
            PAR2 = {s for s, rs in enumerate(ROWS[:-1]) if rs == 32}
            for s, rs in enumerate(ROWS):
                kt = kts[s]
                if s == 3:
                    # HAM keep-alive: junk matmul gated on this tile's DMA
                    # (reads a row the folds never write)
                    nc.tensor.matmul(
                        dummy[0:64, :], lhsT=kt[:, 0, rs - 1, :], rhs=ident,
                        start=True, stop=True, skip_group_check=True,
                    )
                if s in PAR2:
                    ops = []
                    n = rs
                    while n > 4:
                        h = n // 2
                        ops.append((0, h, h, 2 * h))
                        n = h
                    for dlo, dhi, slo, shi in ops:
                        nc.vector.tensor_add(
                            kt[:, :, dlo:dhi, :],
                            kt[:, :, dlo:dhi, :],
                            kt[:, :, slo:shi, :],
                        )
                        if s == 3 and dhi == rs // 2:
                            # second keep-alive, gated on this level-1 output
                            nc.tensor.matmul(
                                dummy[0:64, :], lhsT=kt[:, 0, 12, :], rhs=ident,
                                start=True, stop=True, skip_group_check=True,
                            )
                    par2 = kpool.tile(
                        [128, 2, NPAIR, DIM], FP, tag=f"par{s}", name=f"par{s}"
                    )
                    nc.vector.tensor_add(
                        par2.rearrange("p r q d -> p q r d"),
                        kt[:, :, 0:2, :],
                        kt[:, :, 2:4, :],
                    )
                    for r in range(2):
                        nc.tensor.matmul(
                            PT_ps[:], lhsT=par2[:, r], rhs=lmat_s,
                            start=False,
                            stop=s == nsub - 1 and r == 1,
                            skip_group_check=True,
                        )
                else:
                    ops, (fa, fb) = _chain(rs)
                    for dlo, dhi, slo, shi in ops:
                        nc.vector.tensor_add(
                            kt[:, :, dlo:dhi, :],
                            kt[:, :, dlo:dhi, :],
                            kt[:, :, slo:shi, :],
                        )
                    par = kpool.tile(
                        [128, NPAIR, DIM], FP, tag=f"par{s}", name=f"par{s}"
                    )
                    nc.vector.tensor_add(par[:], kt[:, :, fa, :], kt[:, :, fb, :])
                    nc.tensor.matmul(
                        PT_ps[:], lhsT=par[:], rhs=lmat_s,
                        start=False, stop=s == nsub - 1, skip_group_check=True,
                    )
                if s == 0:
                    # warm-up burst: sustained PE activity flips the HAM clock
                    # gate to 8/8; later matmuls arrive <3.4us apart and hold it
                    for _ in range(32):
                        nc.tensor.matmul(
                            dummy[:], lhsT=cb_sb[:], rhs=cb_sb[:],
                            start=True, stop=True, skip_group_check=True,
                        )

            # ---- tail: PT -> SBUF -> sort projections -> R -> softmax -> out
            PT_sb = small.tile([128, 128], FP, tag="PTs")
            nc.scalar.copy(PT_sb[:], PT_ps[:])
            for p in range(NPAIR):
                prow = slice(64 * p, 64 * p + 64)
                nc.tensor.matmul(
                    SKQs[p][:],
                    lhsT=c64_sb[prow, 104 * p : 104 * p + 104],
                    rhs=PT_sb[prow, :],
                    start=False, stop=True, skip_group_check=True,
                )
            # SQ on scalar, RK on vector: the two copies of each pair overlap
            SQs = []
            RKs = []
            for p in range(NPAIR):
                sq_sb = small.tile([40, 128], FP, tag=f"SQ{p}", name=f"SQ{p}")
                nc.scalar.copy(sq_sb[:], SKQs[p][0:40, :])
                rk_sb = small.tile([40, 128], FP, tag=f"RK{p}", name=f"RK{p}")
                nc.vector.tensor_copy(rk_sb[:], SKQs[p][64:104, :])
                SQs.append(sq_sb)
                RKs.append(rk_sb)
            for p in range(NPAIR):
                nc.tensor.matmul(
                    R_ps[0:64, p, 1:],
                    lhsT=SQs[p][0:8, 0:64],
                    rhs=RKs[p][0:8, 0:64],
                    start=False, stop=False, skip_group_check=True,
                )
                nc.tensor.matmul(
                    R_ps[64:128, p, 1:],
                    lhsT=SQs[p][32:40, 64:128],
                    rhs=RKs[p][32:40, 64:128],
                    start=False, stop=p == NPAIR - 1, skip_group_check=True,
                )

            # masked softmax over 65 logits (zero-logit col 0 is explicit in
            # PSUM from the seed); pair-staggered so pair 0's output DMA is
            # in flight while pair 1 is still in softmax
            mx = small.tile([128, NPAIR], FP, tag="mx")
            nc.vector.reduce_max(mx[:], R_ps[:], axis=X)
            negm = small.tile([128, NPAIR], FP, tag="negm")
            nc.vector.tensor_scalar(
                negm[:], mx[:], 0.0, -1.0,
                op0=mybir.AluOpType.max, op1=MULT,
            )
            e_sb = small.tile([128, NPAIR, BUCKETS + 1], FP, tag="e")
            s1 = small.tile([128, NPAIR], FP, tag="s1")
            rin = small.tile([128, NPAIR], FP, tag="rin")
            outt = small.tile([128, NPAIR, BUCKETS + 1], FP, tag="outt")
            for p in range(NPAIR):
                nc.scalar.activation(
                    e_sb[:, p, :], R_ps[:, p, :], Exp,
                    bias=negm[:, p : p + 1], scale=1.0,
                )
                nc.vector.reduce_sum(s1[:, p : p + 1], e_sb[:, p, :], axis=X)
                nc.vector.reciprocal(rin[:, p : p + 1], s1[:, p : p + 1])
                # outt = (e * 1/den) * tril-mask, fused
                nc.vector.scalar_tensor_tensor(
                    outt[:, p, :],
                    e_sb[:, p, :],
                    rin[:, p : p + 1],
                    mmask,
                    op0=MULT,
                    op1=MULT,
                )
                dst = rout[:, :, p, :].rearrange("b i c -> (b i) c")
                if p == 0:
                    nc.scalar.dma_start(dst, outt[:, p, :])
                else:
                    nc.sync.dma_start(dst, outt[:, p, :])

    nc.compile()
    return nc


def _get_program(enable_asserts=False):
    key = enable_asserts
    if key not in _PROG_CACHE:
        _PROG_CACHE[key] = _build_program(enable_asserts=enable_asserts)
    return _PROG_CACHE[key]


def _host_constants(core, q_pos_emb, k_pos_emb, Wsq, Wsk):
    """Tiny per-core packed constant tensors."""
    f32 = np.float32
    j = np.arange(64, dtype=np.float64)
    s = (1.0 / (CHUNK * j + 1.0)).astype(f32)  # per-bucket cumavg scale

    tri = np.triu(np.ones((64, 64), f32), k=1)  # [c, j] = 1 iff c < j
    tri_s = tri * s[None, :]
    lmat_s = np.zeros((128, 128), f32)
    lmat_s[0:64, 0:64] = tri_s
    lmat_s[64:128, 64:128] = tri_s
    idents = np.zeros((128, 128), f32)
    idents[np.arange(128), np.arange(128)] = np.concatenate([s, s])
    ident = np.eye(128, dtype=f32)

    q = np.arange(64)[:, None]
    jc = np.arange(65)[None, :]
    am = np.where(jc > q, NEG, 0.0).astype(f32)   # softmax additive mask, col0 free
    mm = (jc < q).astype(f32)                     # output tril(-1) mask incl col0
    amask = np.concatenate([am, am], axis=0)      # (128, 65) both b blocks
    mmask = np.concatenate([mm, mm], axis=0)

    c128 = np.concatenate([lmat_s, idents, ident, amask, mmask], axis=1)

    import ml_dtypes

    cb16 = np.eye(128, dtype=ml_dtypes.bfloat16)

    wq_pt = np.zeros((2, 64, 104), f32)   # [pair][d][sq 0:40 | sk 64:104]
    wq_ft = np.zeros((2, 64, 104), f32)
    cblk = np.zeros((2, 104, 128), f32)   # [pair][skq-row][(b, j)]
    for p in range(NPAIR):
        for b in range(2):
            bh = core * BHC + 2 * p + b
            h = bh % HEADS
            r0 = 32 * b
            wq_pt[p, :, r0 : r0 + 8] = Wsq[0, h, 0:64, :]
            wq_pt[p, :, 64 + r0 : 64 + r0 + 8] = Wsk[0, h, 0:64, :]
            wq_ft[p, :, r0 : r0 + 8] = Wsq[0, h, 64:128, :]
            wq_ft[p, :, 64 + r0 : 64 + r0 + 8] = Wsk[0, h, 64:128, :]
            cq = q_pos_emb[0, h] @ Wsq[0, h, 128:192, :]  # (64, 8)
            ck = k_pos_emb[0, h] @ Wsk[0, h, 128:192, :]
            cblk[p, r0 : r0 + 8, 64 * b : 64 * b + 64] = cq.T
            cblk[p, 64 + r0 : 64 + r0 + 8, 64 * b : 64 * b + 64] = ck.T

    c64 = np.concatenate([wq_pt[0], wq_pt[1], wq_ft[0], wq_ft[1]], axis=1)
    c64 = np.concatenate([c64, c64], axis=0)  # duplicate into both halves
    c104 = np.concatenate([cblk[0], cblk[1]], axis=1)
    c104p = np.zeros((128, 256), f32)
    c104p[0:104] = c104
    cM = np.concatenate([c128, c64, c104p], axis=1)
    return {"cM": cM, "cb": cb16}


def _run(k, q_pos_emb, k_pos_emb, Wsq, Wsk, trace=False):
    nc = _get_program()
    in_maps = []
    for core in range(NCORES):
        cm = _host_constants(core, q_pos_emb, k_pos_emb, Wsq, Wsk)
        cm["kin"] = np.ascontiguousarray(k[core * BHC : (core + 1) * BHC])
        in_maps.append(cm)
    res = bass_utils.run_bass_kernel_spmd(
        nc,
        in_maps,
        core_ids=list(range(NCORES)),
        trace=trace,
        **(TRACE_KWARGS if trace else {}),
    )
    global LAST_RESULTS
    LAST_RESULTS = res
    out = np.empty((BH, BUCKETS, BUCKETS + 1), np.float32)
    for core, r in enumerate(res.results):
        ro = r["rout"]  # (2, 64, 2, 65) = (b, i, pair, col)
        for p in range(NPAIR):
            for b in range(2):
                out[core * BHC + 2 * p + b] = ro[b, :, p, :]
    return out, res


def kernel(**inputs):
    k = np.asarray(inputs["k"], np.float32)
    q_pos_emb = np.asarray(inputs["q_pos_emb"], np.float32)
    k_pos_emb = np.asarray(inputs["k_pos_emb"], np.float32)
    Wsq = np.asarray(inputs["Wsq"], np.float32)
    Wsk = np.asarray(inputs["Wsk"], np.float32)
    out, _ = _run(k, q_pos_emb, k_pos_emb, Wsq, Wsk, trace=TRACE)
    return out


# revision 21
# speedup vs baseline: 1.0941x; 1.0941x over previous
"""Trainium2 Bass kernel for CausalAttentionSortNet bucket-scoring.

Math (see reference): only `k` feeds the output. For each merged batch*head
slice, the cumulative-average of k is sampled at bucket starts (every 128th
row), which reduces to per-chunk sums + a strictly-triangular prefix matmul.
The rest is tiny per-bucket sort projections and a 64x65 masked softmax.

Sharding: data-parallel over the merged (batch*heads)=32 axis across 8 cores,
4 slices per core as 2 pairs; partition=(slice_in_pair, chunk), free=(row, dim)
so every partition's k data is one contiguous 32KB HBM run. Both pairs of each
row-group share one SBUF tile so each fold is a single batched instruction.

`q` (half of all input bytes) is never read by the reference computation, so
it is not even transferred to the device.

Per row-group sub-tile: contiguous pairwise fold chains on DVE (unit-stride
fp32 tensor_tensor), with GpSimd taking a d-column share of the first level
in parallel. The folded row 0 feeds the PE prefix matmul (PT). F (row 0 of
chunk) is read straight out of sub-tile 0 before the folds clobber it - no
separate gather DMA. The 64x65 softmax keeps the zero-logit column explicit
in PSUM so the tail is one exp + one scale-mask per pair. Junk matmuls gated
on the stream keep the PE's HAM clock at 8/8 for the tail matmuls.
"""

from contextlib import ExitStack

import numpy as np

import concourse.bacc as bacc
import concourse.mybir as mybir
import concourse.tile as tile
from concourse import bass_utils

# Problem constants (hardcoded per contract; kernel.py must be self-contained).
B, HEADS, BUCKETS, DIM, DIM_SORT, T = 4, 8, 64, 64, 8, 8192
BH = B * HEADS            # 32 merged batch*head slices
NCORES = 8
BHC = BH // NCORES        # 4 slices per core
NPAIR = BHC // 2          # 2 pairs per core
CHUNK = T // BUCKETS      # 128 rows per bucket
NEG = -1.0e30             # softmax mask value (underflows exp to exactly 0)
FP = mybir.dt.float32
BF = mybir.dt.bfloat16

# rows-per-sub-tile (per pair); ascending-then-descending so folds start early
# and the tail tile is tiny. Sum = 128. All folds run on DVE: GpSimd
# tensor_tensor is ~3x slower AND degrades concurrent DVE ops ~4x (measured),
# so handing it any fold share is a net loss.
ROWS = (12, 24, 32, 32, 16, 8, 4)

TRACE = False  # set by test.py for profiling runs
TRACE_KWARGS = {}  # extra run_bass_kernel_spmd kwargs for profiling runs
LAST_RESULTS = None  # BassKernelResults of the most recent run

_PROG_CACHE = {}


def _chain(rs):
    """Pairwise fold schedule for rs rows.

    Returns (ops, final) where ops are in-place (dst_lo, dst_hi, src_lo,
    src_hi) folds and final = (row_a, row_b) whose sum is the column total
    (written to the separate contiguous par tile so it can be a matmul lhsT).
    """
    ops = []
    leftovers = []
    n = rs
    while n > 3:
        h = n // 2
        ops.append((0, h, h, 2 * h))
        if n % 2:
            leftovers.append(n - 1)
        n = h
    if n == 3:
        ops.append((1, 2, 2, 3))
        n = 2
    assert n == 2, rs
    for r in leftovers:
        ops.append((1, 2, r, r + 1))
    return ops, (0, 1)


def _build_program(enable_asserts=False):
    assert sum(ROWS) == CHUNK, (ROWS, CHUNK)
    nsub = len(ROWS)

    nc = bacc.Bacc(
        "TRN2",
        target_bir_lowering=False,
        debug=False,
        enable_asserts=enable_asserts,
        num_devices=NCORES,
    )

    def din(name, shape, dt=FP):
        return nc.dram_tensor(name, shape, dt, kind="ExternalInput").ap()

    kin = din("kin", (BHC, T, DIM))
    # all fp32 constants packed into ONE tensor / ONE DMA (the tile scheduler
    # has only 8 DMA completion-sem lanes; every extra DMA makes a later
    # bulk-DMA issue wait on an in-flight predecessor):
    # cols 0:514    = c128 block [lmat_s | idents | ident | amask65 | mmask65]
    # cols 514:930  = c64 block  [wqk_pt_p0 | wqk_pt_p1 | wqk_ft_p0 | wqk_ft_p1]
    # cols 930:1186 = c104 block (rows 0:104; per pair cq/ck const terms)
    cM = din("cM", (128, 1186))
    # cb: bf16 identity for PE warm-up matmuls
    cb = din("cb", (128, 128), BF)
    # out layout (b, i, pair, col): 520B contiguous per (b, i) partition
    rout = nc.dram_tensor(
        "rout", (2, BUCKETS, NPAIR, BUCKETS + 1), FP, kind="ExternalOutput"
    ).ap()

    Exp = mybir.ActivationFunctionType.Exp
    MULT = mybir.AluOpType.mult
    X = mybir.AxisListType.X

    with tile.TileContext(nc) as tc:
        with ExitStack() as ctx:
            singles = ctx.enter_context(tc.tile_pool(name="singles", bufs=1))
            kpool = ctx.enter_context(tc.tile_pool(name="kpool", bufs=1))
            small = ctx.enter_context(tc.tile_pool(name="small", bufs=2))
            pp = ctx.enter_context(tc.tile_pool(name="pp", bufs=1, space="PSUM"))

            # ---- everything streams on the one sync queue, in priority order:
            # sub-tile 0 (split into half-row DMAs to deepen the early SDMA
            # queue during the bandwidth ramp), then the constants, then the
            # remaining sub-tiles. Pairs share one tile so folds batch both.
            ksrcs = [
                kin[2 * p : 2 * p + 2].rearrange("b (c r) d -> (b c) r d", r=CHUNK)
                for p in range(NPAIR)
            ]
            kts = []
            r0 = 0
            for s, rs in enumerate(ROWS):
                kt = kpool.tile([128, NPAIR, rs, DIM], FP, tag=f"kt{s}")
                for p in range(NPAIR):
                    nc.sync.dma_start(kt[:, p], ksrcs[p][:, r0 : r0 + rs, :])
                kts.append(kt)
                r0 += rs
                if s == 2:
                    # constants are issues #7/#8 on the queue: the tile
                    # scheduler has 8 DMA sem lanes assigned round-robin, so
                    # issue #k blocks on #(k-8)'s completion - with consts
                    # here, the last bulk DMAs' predecessors finish early
                    cb_sb = singles.tile([128, 128], BF, tag="cb")
                    nc.sync.dma_start(cb_sb[:], cb)
                    cM_sb = singles.tile([128, 1186], FP, tag="cM")
                    nc.sync.dma_start(cM_sb[:], cM)

            lmat_s = cM_sb[:, 0:128]
            idents = cM_sb[:, 128:256]
            ident = cM_sb[:, 256:384]
            amask = cM_sb[:, 384 : 384 + 65]
            mmask = cM_sb[:, 449 : 449 + 65]
            c64_sb = cM_sb[:, 514:930]
            c104_sb = cM_sb[0:104, 930:1186]

            # ---- PSUM tiles
            PT_ps = pp.tile([128, 128], FP, tag="PT")
            FT_ps = pp.tile([128, 128], FP, tag="FT")
            SKQs = [
                pp.tile([104, 128], FP, tag=f"SKQ{p}", name=f"SKQ{p}")
                for p in range(NPAIR)
            ]
            R_ps = pp.tile([128, NPAIR, BUCKETS + 1], FP, tag="R")
            dummy = pp.tile([128, 128], FP, tag="dummy")

            # ---- early PE work (consts-gated): R mask seeds + SKQ const seeds
            # one start=True per PSUM bank: start clears has_written bank-wide,
            # so only the first seed may carry it
            for p in range(NPAIR):
                nc.tensor.matmul(
                    R_ps[:, p, :], lhsT=ident, rhs=amask,
                    start=p == 0, stop=False, skip_group_check=True,
                )
            for p in range(NPAIR):
                nc.tensor.matmul(
                    SKQs[p][:], lhsT=ident[0:104, 0:104],
                    rhs=c104_sb[:, 128 * p : 128 * p + 128],
                    start=True, stop=False, skip_group_check=True,
                )

            # ---- F path: copy row 0 of sub-tile 0 to a contiguous tile
            # (matmul lhsT needs one flat free dim) before folds clobber it
            kt0 = kts[0]
            F_sb = small.tile([128, NPAIR, DIM], FP, tag="F")
            nc.vector.tensor_copy(F_sb[:], kt0[:, :, 0, :])
            nc.tensor.matmul(
                PT_ps[:], lhsT=F_sb[:], rhs=idents,
                start=True, stop=False, skip_group_check=True,
            )
            nc.tensor.matmul(
                FT_ps[:], lhsT=F_sb[:], rhs=ident, start=True, stop=True
            )
            FT_sb = small.tile([128, 128], FP, tag="FTs")
            nc.scalar.copy(FT_sb[:], FT_ps[:])
            for p in range(NPAIR):
                prow = slice(64 * p, 64 * p + 64)
                nc.tensor.matmul(
                    SKQs[p][:],
                    lhsT=c64_sb[prow, 208 + 104 * p : 208 + 104 * p + 104],
                    rhs=FT_sb[prow, :],
                    start=False, stop=False, skip_group_check=True,
                )

            # ---- per-sub-tile: batched fold chain then PT accumulation.
            # Power-of-2 tiles stop folding at 4 rows: the last DVE level
            # writes a row-major par2 tile (contiguous (pair, d) rows) and the
            # PE absorbs the final add as two accumulating matmuls.
            PAR2 = {s for s, rs in enumerate(ROWS[:-1]) if rs == 32}
            for s, rs in enumerate(ROWS):
                kt = kts[s]
                if s == 3:
                    # HAM keep-alive: junk matmul gated on this tile's DMA
                    # (reads a row the folds never write)
                    nc.tensor.matmul(
                        dummy[0:64, :], lhsT=kt[:, 0, rs - 1, :], rhs=ident,
                        start=True, stop=True, skip_group_check=True,
                    )
                if s in PAR2:
                    ops = []
                    n = rs
                    while n > 4:
                        h = n // 2
                        ops.append((0, h, h, 2 * h))
                        n = h
                    for dlo, dhi, slo, shi in ops:
                        nc.vector.tensor_add(
                            kt[:, :, dlo:dhi, :],
                            kt[:, :, dlo:dhi, :],
                            kt[:, :, slo:shi, :],
                        )
                        if s == 3 and dhi == rs // 2:
                            # second keep-alive, gated on this level-1 output
                            nc.tensor.matmul(
                                dummy[0:64, :], lhsT=kt[:, 0, 12, :], rhs=ident,
                                start=True, stop=True, skip_group_check=True,
                            )
                    par2 = kpool.tile(
                        [128, 2, NPAIR, DIM], FP, tag=f"par{s}", name=f"par{s}"
                    )
                    nc.vector.tensor_add(
                        par2.rearrange("p r q d -> p q r d"),
                        kt[:, :, 0:2, :],
                        kt[:, :, 2:4, :],
                    )
                    for r in range(2):
                        nc.tensor.matmul(
                            PT_ps[:], lhsT=par2[:, r], rhs=lmat_s,
                            start=False,
                            stop=s == nsub - 1 and r == 1,
                            skip_group_check=True,
                        )
                else:
                    ops, (fa, fb) = _chain(rs)
                    for dlo, dhi, slo, shi in ops:
                        nc.vector.tensor_add(
                            kt[:, :, dlo:dhi, :],
                            kt[:, :, dlo:dhi, :],
                            kt[:, :, slo:shi, :],
                        )
                    par = kpool.tile(
                        [128, NPAIR, DIM], FP, tag=f"par{s}", name=f"par{s}"
                    )
                    nc.vector.tensor_add(par[:], kt[:, :, fa, :], kt[:, :, fb, :])
                    nc.tensor.matmul(
                        PT_ps[:], lhsT=par[:], rhs=lmat_s,
                        start=False, stop=s == nsub - 1, skip_group_check=True,
                    )
                if s == 0:
                    # warm-up burst: sustained PE activity flips the HAM clock
                    # gate to 8/8; later matmuls arrive <3.4us apart and hold it
                    for _ in range(32):
                        nc.tensor.matmul(
                            dummy[:], lhsT=cb_sb[:], rhs=cb_sb[:],
                            start=True, stop=True, skip_group_check=True,
                        )

            # ---- tail: PT -> SBUF -> sort projections -> R -> softmax -> out
            PT_sb = small.tile([128, 128], FP, tag="PTs")
            nc.scalar.copy(PT_sb[:], PT_ps[:])
            for p in range(NPAIR):
                prow = slice(64 * p, 64 * p + 64)
                nc.tensor.matmul(
                    SKQs[p][:],
                    lhsT=c64_sb[prow, 104 * p : 104 * p + 104],
                    rhs=PT_sb[prow, :],
                    start=False, stop=True, skip_group_check=True,
                )
            # SQ on scalar, RK on vector: the two copies of each pair overlap
            SQs = []
            RKs = []
            for p in range(NPAIR):
                sq_sb = small.tile([40, 128], FP, tag=f"SQ{p}", name=f"SQ{p}")
                nc.scalar.copy(sq_sb[:], SKQs[p][0:40, :])
                rk_sb = small.tile([40, 128], FP, tag=f"RK{p}", name=f"RK{p}")
                nc.vector.tensor_copy(rk_sb[:], SKQs[p][64:104, :])
                SQs.append(sq_sb)
                RKs.append(rk_sb)
            for p in range(NPAIR):
                nc.tensor.matmul(
                    R_ps[0:64, p, 1:],
                    lhsT=SQs[p][0:8, 0:64],
                    rhs=RKs[p][0:8, 0:64],
                    start=False, stop=False, skip_group_check=True,
                )
                nc.tensor.matmul(
                    R_ps[64:128, p, 1:],
                    lhsT=SQs[p][32:40, 64:128],
                    rhs=RKs[p][32:40, 64:128],
                    start=False, stop=p == NPAIR - 1, skip_group_check=True,
                )

            # masked softmax over 65 logits (zero-logit col 0 is explicit in
            # PSUM from the seed); pair-staggered so pair 0's output DMA is
            # in flight while pair 1 is still in softmax
            mx = small.tile([128, NPAIR], FP, tag="mx")
            nc.vector.reduce_max(mx[:], R_ps[:], axis=X)
            negm = small.tile([128, NPAIR], FP, tag="negm")
            nc.vector.tensor_scalar(
                negm[:], mx[:], 0.0, -1.0,
                op0=mybir.AluOpType.max, op1=MULT,
            )
            e_sb = small.tile([128, NPAIR, BUCKETS + 1], FP, tag="e")
            s1 = small.tile([128, NPAIR], FP, tag="s1")
            rin = small.tile([128, NPAIR], FP, tag="rin")
            outt = small.tile([128, NPAIR, BUCKETS + 1], FP, tag="outt")
            for p in range(NPAIR):
                nc.scalar.activation(
                    e_sb[:, p, :], R_ps[:, p, :], Exp,
                    bias=negm[:, p : p + 1], scale=1.0,
                )
                nc.vector.reduce_sum(s1[:, p : p + 1], e_sb[:, p, :], axis=X)
                nc.vector.reciprocal(rin[:, p : p + 1], s1[:, p : p + 1])
                # outt = (e * 1/den) * tril-mask, fused
                nc.vector.scalar_tensor_tensor(
                    outt[:, p, :],
                    e_sb[:, p, :],
                    rin[:, p : p + 1],
                    mmask,
                    op0=MULT,
                    op1=MULT,
                )
                dst = rout[:, :, p, :].rearrange("b i c -> (b i) c")
                if p == 0:
                    nc.scalar.dma_start(dst, outt[:, p, :])
                else:
                    nc.sync.dma_start(dst, outt[:, p, :])

    nc.compile()
    return nc


def _get_program(enable_asserts=False):
    key = enable_asserts
    if key not in _PROG_CACHE:
        _PROG_CACHE[key] = _build_program(enable_asserts=enable_asserts)
    return _PROG_CACHE[key]


def _host_constants(core, q_pos_emb, k_pos_emb, Wsq, Wsk):
    """Tiny per-core packed constant tensors."""
    f32 = np.float32
    j = np.arange(64, dtype=np.float64)
    s = (1.0 / (CHUNK * j + 1.0)).astype(f32)  # per-bucket cumavg scale

    tri = np.triu(np.ones((64, 64), f32), k=1)  # [c, j] = 1 iff c < j
    tri_s = tri * s[None, :]
    lmat_s = np.zeros((128, 128), f32)
    lmat_s[0:64, 0:64] = tri_s
    lmat_s[64:128, 64:128] = tri_s
    idents = np.zeros((128, 128), f32)
    idents[np.arange(128), np.arange(128)] = np.concatenate([s, s])
    ident = np.eye(128, dtype=f32)

    q = np.arange(64)[:, None]
    jc = np.arange(65)[None, :]
    am = np.where(jc > q, NEG, 0.0).astype(f32)   # softmax additive mask, col0 free
    mm = (jc < q).astype(f32)                     # output tril(-1) mask incl col0
    amask = np.concatenate([am, am], axis=0)      # (128, 65) both b blocks
    mmask = np.concatenate([mm, mm], axis=0)

    c128 = np.concatenate([lmat_s, idents, ident, amask, mmask], axis=1)

    import ml_dtypes

    cb16 = np.eye(128, dtype=ml_dtypes.bfloat16)

    wq_pt = np.zeros((2, 64, 104), f32)   # [pair][d][sq 0:40 | sk 64:104]
    wq_ft = np.zeros((2, 64, 104), f32)
    cblk = np.zeros((2, 104, 128), f32)   # [pair][skq-row][(b, j)]
    for p in range(NPAIR):
        for b in range(2):
            bh = core * BHC + 2 * p + b
            h = bh % HEADS
            r0 = 32 * b
            wq_pt[p, :, r0 : r0 + 8] = Wsq[0, h, 0:64, :]
            wq_pt[p, :, 64 + r0 : 64 + r0 + 8] = Wsk[0, h, 0:64, :]
            wq_ft[p, :, r0 : r0 + 8] = Wsq[0, h, 64:128, :]
            wq_ft[p, :, 64 + r0 : 64 + r0 + 8] = Wsk[0, h, 64:128, :]
            cq = q_pos_emb[0, h] @ Wsq[0, h, 128:192, :]  # (64, 8)
            ck = k_pos_emb[0, h] @ Wsk[0, h, 128:192, :]
            cblk[p, r0 : r0 + 8, 64 * b : 64 * b + 64] = cq.T
            cblk[p, 64 + r0 : 64 + r0 + 8, 64 * b : 64 * b + 64] = ck.T

    c64 = np.concatenate([wq_pt[0], wq_pt[1], wq_ft[0], wq_ft[1]], axis=1)
    c64 = np.concatenate([c64, c64], axis=0)  # duplicate into both halves
    c104 = np.concatenate([cblk[0], cblk[1]], axis=1)
    c104p = np.zeros((128, 256), f32)
    c104p[0:104] = c104
    cM = np.concatenate([c128, c64, c104p], axis=1)
    return {"cM": cM, "cb": cb16}


def _run(k, q_pos_emb, k_pos_emb, Wsq, Wsk, trace=False):
    nc = _get_program()
    in_maps = []
    for core in range(NCORES):
        cm = _host_constants(core, q_pos_emb, k_pos_emb, Wsq, Wsk)
        cm["kin"] = np.ascontiguousarray(k[core * BHC : (core + 1) * BHC])
        in_maps.append(cm)
    res = bass_utils.run_bass_kernel_spmd(
        nc,
        in_maps,
        core_ids=list(range(NCORES)),
        trace=trace,
        **(TRACE_KWARGS if trace else {}),
    )
    global LAST_RESULTS
    LAST_RESULTS = res
    out = np.empty((BH, BUCKETS, BUCKETS + 1), np.float32)
    for core, r in enumerate(res.results):
        ro = r["rout"]  # (2, 64, 2, 65) = (b, i, pair, col)
        for p in range(NPAIR):
            for b in range(2):
                out[core * BHC + 2 * p + b] = ro[b, :, p, :]
    return out, res


def kernel(**inputs):
    k = np.asarray(inputs["k"], np.float32)
    q_pos_emb = np.asarray(inputs["q_pos_emb"], np.float32)
    k_pos_emb = np.asarray(inputs["k_pos_emb"], np.float32)
    Wsq = np.asarray(inputs["Wsq"], np.float32)
    Wsk = np.asarray(inputs["Wsk"], np.float32)
    out, _ = _run(k, q_pos_emb, k_pos_emb, Wsq, Wsk, trace=TRACE)
    return out


# revision 22
# speedup vs baseline: 1.1814x; 1.0798x over previous
"""Trainium2 Bass kernel for CausalAttentionSortNet bucket-scoring.

Math (see reference): only `k` feeds the output. For each merged batch*head
slice, the cumulative-average of k is sampled at bucket starts (every 128th
row), which reduces to per-chunk sums + a strictly-triangular prefix matmul.
The rest is tiny per-bucket sort projections and a 64x65 masked softmax.

Sharding: data-parallel over the merged (batch*heads)=32 axis across 8 cores,
4 slices per core as 2 pairs; partition=(slice_in_pair, chunk), free=(row, dim)
so every partition's k data is one contiguous 32KB HBM run. Both pairs of each
row-group share one SBUF tile so each fold is a single batched instruction.

`q` (half of all input bytes) is never read by the reference computation, so
it is not even transferred to the device.

Per row-group sub-tile: contiguous pairwise fold chains on DVE (unit-stride
fp32 tensor_tensor), with GpSimd taking a d-column share of the first level
in parallel. The folded row 0 feeds the PE prefix matmul (PT). F (row 0 of
chunk) is read straight out of sub-tile 0 before the folds clobber it - no
separate gather DMA. The 64x65 softmax keeps the zero-logit column explicit
in PSUM so the tail is one exp + one scale-mask per pair. Junk matmuls gated
on the stream keep the PE's HAM clock at 8/8 for the tail matmuls.
"""

from contextlib import ExitStack

import numpy as np

import concourse.bacc as bacc
import concourse.mybir as mybir
import concourse.tile as tile
from concourse import bass_utils

# Problem constants (hardcoded per contract; kernel.py must be self-contained).
B, HEADS, BUCKETS, DIM, DIM_SORT, T = 4, 8, 64, 64, 8, 8192
BH = B * HEADS            # 32 merged batch*head slices
NCORES = 8
BHC = BH // NCORES        # 4 slices per core
NPAIR = BHC // 2          # 2 pairs per core
CHUNK = T // BUCKETS      # 128 rows per bucket
NEG = -1.0e30             # softmax mask value (underflows exp to exactly 0)
FP = mybir.dt.float32
BF = mybir.dt.bfloat16

# rows-per-sub-tile (per pair). Sum = 128. Per-partition DMA descriptor size
# is rows*256B and SDMA engines need ~8KB descriptors for full rate, so the
# stream leads with 32-row tiles; the tail tiles shrink so the last fold
# chain is short. All folds run on DVE: GpSimd tensor_tensor is ~3x slower
# AND degrades concurrent DVE ops ~4x (measured), so it gets no share.
ROWS = (32, 32, 32, 16, 12, 4)

TRACE = False  # set by test.py for profiling runs
TRACE_KWARGS = {}  # extra run_bass_kernel_spmd kwargs for profiling runs
LAST_RESULTS = None  # BassKernelResults of the most recent run

_PROG_CACHE = {}


def _chain(rs):
    """Pairwise fold schedule for rs rows.

    Returns (ops, final) where ops are in-place (dst_lo, dst_hi, src_lo,
    src_hi) folds and final = (row_a, row_b) whose sum is the column total
    (written to the separate contiguous par tile so it can be a matmul lhsT).
    """
    ops = []
    leftovers = []
    n = rs
    while n > 3:
        h = n // 2
        ops.append((0, h, h, 2 * h))
        if n % 2:
            leftovers.append(n - 1)
        n = h
    if n == 3:
        ops.append((1, 2, 2, 3))
        n = 2
    assert n == 2, rs
    for r in leftovers:
        ops.append((1, 2, r, r + 1))
    return ops, (0, 1)


def _build_program(enable_asserts=False):
    assert sum(ROWS) == CHUNK, (ROWS, CHUNK)
    nsub = len(ROWS)

    nc = bacc.Bacc(
        "TRN2",
        target_bir_lowering=False,
        debug=False,
        enable_asserts=enable_asserts,
        num_devices=NCORES,
    )

    def din(name, shape, dt=FP):
        return nc.dram_tensor(name, shape, dt, kind="ExternalInput").ap()

    kin = din("kin", (BHC, T, DIM))
    # all fp32 constants packed into ONE tensor / ONE DMA (the tile scheduler
    # has only 8 DMA completion-sem lanes; every extra DMA makes a later
    # bulk-DMA issue wait on an in-flight predecessor):
    # cols 0:514    = c128 block [lmat_s | idents | ident | amask65 | mmask65]
    # cols 514:930  = c64 block  [wqk_pt_p0 | wqk_pt_p1 | wqk_ft_p0 | wqk_ft_p1]
    # cols 930:1186 = c104 block (rows 0:104; per pair cq/ck const terms)
    cM = din("cM", (128, 1186))
    # cb: bf16 identity for PE warm-up matmuls
    cb = din("cb", (128, 128), BF)
    # out layout (b, i, pair, col): 520B contiguous per (b, i) partition
    rout = nc.dram_tensor(
        "rout", (2, BUCKETS, NPAIR, BUCKETS + 1), FP, kind="ExternalOutput"
    ).ap()

    Exp = mybir.ActivationFunctionType.Exp
    MULT = mybir.AluOpType.mult
    X = mybir.AxisListType.X

    with tile.TileContext(nc) as tc:
        with ExitStack() as ctx:
            singles = ctx.enter_context(tc.tile_pool(name="singles", bufs=1))
            kpool = ctx.enter_context(tc.tile_pool(name="kpool", bufs=1))
            small = ctx.enter_context(tc.tile_pool(name="small", bufs=2))
            pp = ctx.enter_context(tc.tile_pool(name="pp", bufs=1, space="PSUM"))

            # ---- everything streams on the one sync queue, in priority order:
            # sub-tile 0 (split into half-row DMAs to deepen the early SDMA
            # queue during the bandwidth ramp), then the constants, then the
            # remaining sub-tiles. Pairs share one tile so folds batch both.
            ksrcs = [
                kin[2 * p : 2 * p + 2].rearrange("b (c r) d -> (b c) r d", r=CHUNK)
                for p in range(NPAIR)
            ]
            kts = []
            r0 = 0
            for s, rs in enumerate(ROWS):
                kt = kpool.tile([128, NPAIR, rs, DIM], FP, tag=f"kt{s}")
                for p in range(NPAIR):
                    nc.sync.dma_start(kt[:, p], ksrcs[p][:, r0 : r0 + rs, :])
                kts.append(kt)
                r0 += rs
                if s == 0:
                    # constants ride behind sub-tile 0; with 16 total DMAs and
                    # 8 round-robin sem lanes, issue #k blocks on #(k-8)'s
                    # completion - all predecessors here complete early
                    cb_sb = singles.tile([128, 128], BF, tag="cb")
                    nc.sync.dma_start(cb_sb[:], cb)
                    cM_sb = singles.tile([128, 1186], FP, tag="cM")
                    nc.sync.dma_start(cM_sb[:], cM)

            lmat_s = cM_sb[:, 0:128]
            idents = cM_sb[:, 128:256]
            ident = cM_sb[:, 256:384]
            amask = cM_sb[:, 384 : 384 + 65]
            mmask = cM_sb[:, 449 : 449 + 65]
            c64_sb = cM_sb[:, 514:930]
            c104_sb = cM_sb[0:104, 930:1186]

            # ---- PSUM tiles
            PT_ps = pp.tile([128, 128], FP, tag="PT")
            FT_ps = pp.tile([128, 128], FP, tag="FT")
            SKQs = [
                pp.tile([104, 128], FP, tag=f"SKQ{p}", name=f"SKQ{p}")
                for p in range(NPAIR)
            ]
            R_ps = pp.tile([128, NPAIR, BUCKETS + 1], FP, tag="R")
            dummy = pp.tile([128, 128], FP, tag="dummy")

            # ---- early PE work (consts-gated): R mask seeds + SKQ const seeds
            # one start=True per PSUM bank: start clears has_written bank-wide,
            # so only the first seed may carry it
            for p in range(NPAIR):
                nc.tensor.matmul(
                    R_ps[:, p, :], lhsT=ident, rhs=amask,
                    start=p == 0, stop=False, skip_group_check=True,
                )
            for p in range(NPAIR):
                nc.tensor.matmul(
                    SKQs[p][:], lhsT=ident[0:104, 0:104],
                    rhs=c104_sb[:, 128 * p : 128 * p + 128],
                    start=True, stop=False, skip_group_check=True,
                )

            # ---- F path: copy row 0 of sub-tile 0 to a contiguous tile
            # (matmul lhsT needs one flat free dim) before folds clobber it
            kt0 = kts[0]
            F_sb = small.tile([128, NPAIR, DIM], FP, tag="F")
            nc.vector.tensor_copy(F_sb[:], kt0[:, :, 0, :])
            nc.tensor.matmul(
                PT_ps[:], lhsT=F_sb[:], rhs=idents,
                start=True, stop=False, skip_group_check=True,
            )
            nc.tensor.matmul(
                FT_ps[:], lhsT=F_sb[:], rhs=ident, start=True, stop=True
            )
            FT_sb = small.tile([128, 128], FP, tag="FTs")
            nc.scalar.copy(FT_sb[:], FT_ps[:])
            for p in range(NPAIR):
                prow = slice(64 * p, 64 * p + 64)
                nc.tensor.matmul(
                    SKQs[p][:],
                    lhsT=c64_sb[prow, 208 + 104 * p : 208 + 104 * p + 104],
                    rhs=FT_sb[prow, :],
                    start=False, stop=False, skip_group_check=True,
                )

            # ---- per-sub-tile: batched fold chain then PT accumulation.
            # Power-of-2 tiles stop folding at 4 rows: the last DVE level
            # writes a row-major par2 tile (contiguous (pair, d) rows) and the
            # PE absorbs the final add as two accumulating matmuls.
            PAR2 = {s for s, rs in enumerate(ROWS[:-1]) if rs in (16, 32)}
            for s, rs in enumerate(ROWS):
                kt = kts[s]
                if s in PAR2:
                    ops = []
                    n = rs
                    while n > 4:
                        h = n // 2
                        ops.append((0, h, h, 2 * h))
                        n = h
                    for dlo, dhi, slo, shi in ops:
                        nc.vector.tensor_add(
                            kt[:, :, dlo:dhi, :],
                            kt[:, :, dlo:dhi, :],
                            kt[:, :, slo:shi, :],
                        )
                    par2 = kpool.tile(
                        [128, 2, NPAIR, DIM], FP, tag=f"par{s}", name=f"par{s}"
                    )
                    nc.vector.tensor_add(
                        par2.rearrange("p r q d -> p q r d"),
                        kt[:, :, 0:2, :],
                        kt[:, :, 2:4, :],
                    )
                    for r in range(2):
                        nc.tensor.matmul(
                            PT_ps[:], lhsT=par2[:, r], rhs=lmat_s,
                            start=False,
                            stop=s == nsub - 1 and r == 1,
                            skip_group_check=True,
                        )
                else:
                    ops, (fa, fb) = _chain(rs)
                    for dlo, dhi, slo, shi in ops:
                        nc.vector.tensor_add(
                            kt[:, :, dlo:dhi, :],
                            kt[:, :, dlo:dhi, :],
                            kt[:, :, slo:shi, :],
                        )
                    par = kpool.tile(
                        [128, NPAIR, DIM], FP, tag=f"par{s}", name=f"par{s}"
                    )
                    nc.vector.tensor_add(par[:], kt[:, :, fa, :], kt[:, :, fb, :])
                    nc.tensor.matmul(
                        PT_ps[:], lhsT=par[:], rhs=lmat_s,
                        start=False, stop=s == nsub - 1, skip_group_check=True,
                    )
                if s == 0:
                    # warm-up burst: sustained PE activity flips the HAM clock
                    # gate to 8/8; later matmuls arrive <3.4us apart and hold it
                    for _ in range(32):
                        nc.tensor.matmul(
                            dummy[:], lhsT=cb_sb[:], rhs=cb_sb[:],
                            start=True, stop=True, skip_group_check=True,
                        )

            # ---- tail: PT -> SBUF -> sort projections -> R -> softmax -> out
            PT_sb = small.tile([128, 128], FP, tag="PTs")
            nc.vector.tensor_copy(PT_sb[:], PT_ps[:])
            for p in range(NPAIR):
                prow = slice(64 * p, 64 * p + 64)
                nc.tensor.matmul(
                    SKQs[p][:],
                    lhsT=c64_sb[prow, 104 * p : 104 * p + 104],
                    rhs=PT_sb[prow, :],
                    start=False, stop=True, skip_group_check=True,
                )
            # SQ on scalar, RK on vector: the two copies of each pair overlap
            SQs = []
            RKs = []
            for p in range(NPAIR):
                sq_sb = small.tile([40, 128], FP, tag=f"SQ{p}", name=f"SQ{p}")
                nc.scalar.copy(sq_sb[:], SKQs[p][0:40, :])
                rk_sb = small.tile([40, 128], FP, tag=f"RK{p}", name=f"RK{p}")
                nc.vector.tensor_copy(rk_sb[:], SKQs[p][64:104, :])
                SQs.append(sq_sb)
                RKs.append(rk_sb)
            for p in range(NPAIR):
                nc.tensor.matmul(
                    R_ps[0:64, p, 1:],
                    lhsT=SQs[p][0:8, 0:64],
                    rhs=RKs[p][0:8, 0:64],
                    start=False, stop=False, skip_group_check=True,
                )
                nc.tensor.matmul(
                    R_ps[64:128, p, 1:],
                    lhsT=SQs[p][32:40, 64:128],
                    rhs=RKs[p][32:40, 64:128],
                    start=False, stop=p == NPAIR - 1, skip_group_check=True,
                )

            # masked softmax over 65 logits (zero-logit col 0 is explicit in
            # PSUM from the seed); pair-staggered so pair 0's output DMA is
            # in flight while pair 1 is still in softmax
            mx = small.tile([128, NPAIR], FP, tag="mx")
            nc.vector.reduce_max(mx[:], R_ps[:], axis=X)
            negm = small.tile([128, NPAIR], FP, tag="negm")
            nc.vector.tensor_scalar(
                negm[:], mx[:], 0.0, -1.0,
                op0=mybir.AluOpType.max, op1=MULT,
            )
            e_sb = small.tile([128, NPAIR, BUCKETS + 1], FP, tag="e")
            s1 = small.tile([128, NPAIR], FP, tag="s1")
            rin = small.tile([128, NPAIR], FP, tag="rin")
            outt = small.tile([128, NPAIR, BUCKETS + 1], FP, tag="outt")
            for p in range(NPAIR):
                nc.scalar.activation(
                    e_sb[:, p, :], R_ps[:, p, :], Exp,
                    bias=negm[:, p : p + 1], scale=1.0,
                )
                nc.vector.reduce_sum(s1[:, p : p + 1], e_sb[:, p, :], axis=X)
                nc.vector.reciprocal(rin[:, p : p + 1], s1[:, p : p + 1])
                # outt = (e * 1/den) * tril-mask, fused
                nc.vector.scalar_tensor_tensor(
                    outt[:, p, :],
                    e_sb[:, p, :],
                    rin[:, p : p + 1],
                    mmask,
                    op0=MULT,
                    op1=MULT,
                )
                dst = rout[:, :, p, :].rearrange("b i c -> (b i) c")
                if p == 0:
                    nc.scalar.dma_start(dst, outt[:, p, :])
                else:
                    nc.sync.dma_start(dst, outt[:, p, :])

    nc.compile()
    return nc


def _get_program(enable_asserts=False):
    key = enable_asserts
    if key not in _PROG_CACHE:
        _PROG_CACHE[key] = _build_program(enable_asserts=enable_asserts)
    return _PROG_CACHE[key]


def _host_constants(core, q_pos_emb, k_pos_emb, Wsq, Wsk):
    """Tiny per-core packed constant tensors."""
    f32 = np.float32
    j = np.arange(64, dtype=np.float64)
    s = (1.0 / (CHUNK * j + 1.0)).astype(f32)  # per-bucket cumavg scale

    tri = np.triu(np.ones((64, 64), f32), k=1)  # [c, j] = 1 iff c < j
    tri_s = tri * s[None, :]
    lmat_s = np.zeros((128, 128), f32)
    lmat_s[0:64, 0:64] = tri_s
    lmat_s[64:128, 64:128] = tri_s
    idents = np.zeros((128, 128), f32)
    idents[np.arange(128), np.arange(128)] = np.concatenate([s, s])
    ident = np.eye(128, dtype=f32)

    q = np.arange(64)[:, None]
    jc = np.arange(65)[None, :]
    am = np.where(jc > q, NEG, 0.0).astype(f32)   # softmax additive mask, col0 free
    mm = (jc < q).astype(f32)                     # output tril(-1) mask incl col0
    amask = np.concatenate([am, am], axis=0)      # (128, 65) both b blocks
    mmask = np.concatenate([mm, mm], axis=0)

    c128 = np.concatenate([lmat_s, idents, ident, amask, mmask], axis=1)

    import ml_dtypes

    cb16 = np.eye(128, dtype=ml_dtypes.bfloat16)

    wq_pt = np.zeros((2, 64, 104), f32)   # [pair][d][sq 0:40 | sk 64:104]
    wq_ft = np.zeros((2, 64, 104), f32)
    cblk = np.zeros((2, 104, 128), f32)   # [pair][skq-row][(b, j)]
    for p in range(NPAIR):
        for b in range(2):
            bh = core * BHC + 2 * p + b
            h = bh % HEADS
            r0 = 32 * b
            wq_pt[p, :, r0 : r0 + 8] = Wsq[0, h, 0:64, :]
            wq_pt[p, :, 64 + r0 : 64 + r0 + 8] = Wsk[0, h, 0:64, :]
            wq_ft[p, :, r0 : r0 + 8] = Wsq[0, h, 64:128, :]
            wq_ft[p, :, 64 + r0 : 64 + r0 + 8] = Wsk[0, h, 64:128, :]
            cq = q_pos_emb[0, h] @ Wsq[0, h, 128:192, :]  # (64, 8)
            ck = k_pos_emb[0, h] @ Wsk[0, h, 128:192, :]
            cblk[p, r0 : r0 + 8, 64 * b : 64 * b + 64] = cq.T
            cblk[p, 64 + r0 : 64 + r0 + 8, 64 * b : 64 * b + 64] = ck.T

    c64 = np.concatenate([wq_pt[0], wq_pt[1], wq_ft[0], wq_ft[1]], axis=1)
    c64 = np.concatenate([c64, c64], axis=0)  # duplicate into both halves
    c104 = np.concatenate([cblk[0], cblk[1]], axis=1)
    c104p = np.zeros((128, 256), f32)
    c104p[0:104] = c104
    cM = np.concatenate([c128, c64, c104p], axis=1)
    return {"cM": cM, "cb": cb16}


def _run(k, q_pos_emb, k_pos_emb, Wsq, Wsk, trace=False):
    nc = _get_program()
    in_maps = []
    for core in range(NCORES):
        cm = _host_constants(core, q_pos_emb, k_pos_emb, Wsq, Wsk)
        cm["kin"] = np.ascontiguousarray(k[core * BHC : (core + 1) * BHC])
        in_maps.append(cm)
    res = bass_utils.run_bass_kernel_spmd(
        nc,
        in_maps,
        core_ids=list(range(NCORES)),
        trace=trace,
        **(TRACE_KWARGS if trace else {}),
    )
    global LAST_RESULTS
    LAST_RESULTS = res
    out = np.empty((BH, BUCKETS, BUCKETS + 1), np.float32)
    for core, r in enumerate(res.results):
        ro = r["rout"]  # (2, 64, 2, 65) = (b, i, pair, col)
        for p in range(NPAIR):
            for b in range(2):
                out[core * BHC + 2 * p + b] = ro[b, :, p, :]
    return out, res


def kernel(**inputs):
    k = np.asarray(inputs["k"], np.float32)
    q_pos_emb = np.asarray(inputs["q_pos_emb"], np.float32)
    k_pos_emb = np.asarray(inputs["k_pos_emb"], np.float32)
    Wsq = np.asarray(inputs["Wsq"], np.float32)
    Wsk = np.asarray(inputs["Wsk"], np.float32)
    out, _ = _run(k, q_pos_emb, k_pos_emb, Wsq, Wsk, trace=TRACE)
    return out


# revision 23
# speedup vs baseline: 1.2424x; 1.0516x over previous
"""Trainium2 Bass kernel for CausalAttentionSortNet bucket-scoring.

Math (see reference): only `k` feeds the output. For each merged batch*head
slice, the cumulative-average of k is sampled at bucket starts (every 128th
row), which reduces to per-chunk sums + a strictly-triangular prefix matmul.
The rest is tiny per-bucket sort projections and a 64x65 masked softmax.

Sharding: data-parallel over the merged (batch*heads)=32 axis across 8 cores,
4 slices per core as 2 pairs; partition=(slice_in_pair, chunk), free=(row, dim)
so every partition's k data is one contiguous 32KB HBM run. Both pairs of each
row-group share one SBUF tile so each fold is a single batched instruction.

`q` (half of all input bytes) is never read by the reference computation, so
it is not even transferred to the device.

Per row-group sub-tile: contiguous pairwise fold chains on DVE (unit-stride
fp32 tensor_tensor), with GpSimd taking a d-column share of the first level
in parallel. The folded row 0 feeds the PE prefix matmul (PT). F (row 0 of
chunk) is read straight out of sub-tile 0 before the folds clobber it - no
separate gather DMA. The 64x65 softmax keeps the zero-logit column explicit
in PSUM so the tail is one exp + one scale-mask per pair. Junk matmuls gated
on the stream keep the PE's HAM clock at 8/8 for the tail matmuls.
"""

from contextlib import ExitStack

import numpy as np

import concourse.bacc as bacc
import concourse.mybir as mybir
import concourse.tile as tile
from concourse import bass_utils

# Problem constants (hardcoded per contract; kernel.py must be self-contained).
B, HEADS, BUCKETS, DIM, DIM_SORT, T = 4, 8, 64, 64, 8, 8192
BH = B * HEADS            # 32 merged batch*head slices
NCORES = 8
BHC = BH // NCORES        # 4 slices per core
NPAIR = BHC // 2          # 2 pairs per core
CHUNK = T // BUCKETS      # 128 rows per bucket
NEG = -1.0e30             # softmax mask value (underflows exp to exactly 0)
FP = mybir.dt.float32
BF = mybir.dt.bfloat16
F16 = mybir.dt.float16

# rows-per-sub-tile (per pair). Sum = 128. Per-partition DMA descriptor size
# is rows*256B and SDMA engines need ~8KB descriptors for full rate, so the
# stream leads with 32-row tiles; the tail tiles shrink so the last fold
# chain is short. All folds run on DVE: GpSimd tensor_tensor is ~3x slower
# AND degrades concurrent DVE ops ~4x (measured), so it gets no share.
ROWS = (32, 32, 32, 16, 12, 4)

TRACE = False  # set by test.py for profiling runs
TRACE_KWARGS = {}  # extra run_bass_kernel_spmd kwargs for profiling runs
LAST_RESULTS = None  # BassKernelResults of the most recent run

_PROG_CACHE = {}


def _chain(rs):
    """Pairwise fold schedule for rs rows.

    Returns (ops, final) where ops are in-place (dst_lo, dst_hi, src_lo,
    src_hi) folds and final = (row_a, row_b) whose sum is the column total
    (written to the separate contiguous par tile so it can be a matmul lhsT).
    """
    ops = []
    leftovers = []
    n = rs
    while n > 3:
        h = n // 2
        ops.append((0, h, h, 2 * h))
        if n % 2:
            leftovers.append(n - 1)
        n = h
    if n == 3:
        ops.append((1, 2, 2, 3))
        n = 2
    assert n == 2, rs
    for r in leftovers:
        ops.append((1, 2, r, r + 1))
    return ops, (0, 1)


def _build_program(enable_asserts=False):
    assert sum(ROWS) == CHUNK, (ROWS, CHUNK)
    nsub = len(ROWS)

    nc = bacc.Bacc(
        "TRN2",
        target_bir_lowering=False,
        debug=False,
        enable_asserts=enable_asserts,
        num_devices=NCORES,
    )

    def din(name, shape, dt=FP):
        return nc.dram_tensor(name, shape, dt, kind="ExternalInput").ap()

    kin = din("kin", (BHC, T, DIM))
    # all fp32 constants packed into ONE tensor / ONE DMA (the tile scheduler
    # has only 8 DMA completion-sem lanes; every extra DMA makes a later
    # bulk-DMA issue wait on an in-flight predecessor):
    # cols 0:514    = c128 block [lmat_s | idents | ident | amask65 | mmask65]
    # cols 514:930  = c64 block  [wqk_pt_p0 | wqk_pt_p1 | wqk_ft_p0 | wqk_ft_p1]
    # cols 930:1186 = c104 block (rows 0:104; per pair cq/ck const terms)
    cM = din("cM", (128, 1186))
    # cb: bf16 identity for PE warm-up matmuls
    cb = din("cb", (128, 128), BF)
    # c16: the scaled prefix-tril matrix in fp16 (the whole chunk-sum path
    # runs in fp16: rel-err 3.8e-3 vs the 2e-2 gate; bf16 fails at 2.1e-2)
    c16 = din("c16", (128, 128), F16)
    # out layout (b, i, pair, col): 520B contiguous per (b, i) partition
    rout = nc.dram_tensor(
        "rout", (2, BUCKETS, NPAIR, BUCKETS + 1), FP, kind="ExternalOutput"
    ).ap()

    Exp = mybir.ActivationFunctionType.Exp
    MULT = mybir.AluOpType.mult
    X = mybir.AxisListType.X

    with tile.TileContext(nc) as tc:
        with ExitStack() as ctx:
            singles = ctx.enter_context(tc.tile_pool(name="singles", bufs=1))
            kpool = ctx.enter_context(tc.tile_pool(name="kpool", bufs=1))
            small = ctx.enter_context(tc.tile_pool(name="small", bufs=2))
            pp = ctx.enter_context(tc.tile_pool(name="pp", bufs=1, space="PSUM"))

            # ---- everything streams on the one sync queue, in priority order:
            # sub-tile 0 (split into half-row DMAs to deepen the early SDMA
            # queue during the bandwidth ramp), then the constants, then the
            # remaining sub-tiles. Pairs share one tile so folds batch both.
            ksrcs = [
                kin[2 * p : 2 * p + 2].rearrange("b (c r) d -> (b c) r d", r=CHUNK)
                for p in range(NPAIR)
            ]
            kts = []
            r0 = 0
            for s, rs in enumerate(ROWS):
                kt = kpool.tile([128, NPAIR, rs, DIM], FP, tag=f"kt{s}")
                for p in range(NPAIR):
                    nc.sync.dma_start(kt[:, p], ksrcs[p][:, r0 : r0 + rs, :])
                kts.append(kt)
                r0 += rs
                if s == 0:
                    # constants ride behind sub-tile 0; with 16 total DMAs and
                    # 8 round-robin sem lanes, issue #k blocks on #(k-8)'s
                    # completion - all predecessors here complete early
                    cb_sb = singles.tile([128, 128], BF, tag="cb")
                    nc.sync.dma_start(cb_sb[:], cb)
                    cM_sb = singles.tile([128, 1186], FP, tag="cM")
                    nc.sync.dma_start(cM_sb[:], cM)
                    c16_sb = singles.tile([128, 128], F16, tag="c16")
                    nc.sync.dma_start(c16_sb[:], c16)

            lmat_s = c16_sb[:]
            idents = cM_sb[:, 128:256]
            ident = cM_sb[:, 256:384]
            amask = cM_sb[:, 384 : 384 + 65]
            mmask = cM_sb[:, 449 : 449 + 65]
            c64_sb = cM_sb[:, 514:930]
            c104_sb = cM_sb[0:104, 930:1186]

            # ---- PSUM tiles
            PT_ps = pp.tile([128, 128], FP, tag="PT")
            FT_ps = pp.tile([128, 128], FP, tag="FT")
            SKQs = [
                pp.tile([104, 128], FP, tag=f"SKQ{p}", name=f"SKQ{p}")
                for p in range(NPAIR)
            ]
            R_ps = pp.tile([128, NPAIR, BUCKETS + 1], FP, tag="R")
            dummy = pp.tile([128, 128], FP, tag="dummy")

            # ---- early PE work (consts-gated): R mask seeds + SKQ const seeds
            # one start=True per PSUM bank: start clears has_written bank-wide,
            # so only the first seed may carry it
            for p in range(NPAIR):
                nc.tensor.matmul(
                    R_ps[:, p, :], lhsT=ident, rhs=amask,
                    start=p == 0, stop=False, skip_group_check=True,
                )
            for p in range(NPAIR):
                nc.tensor.matmul(
                    SKQs[p][:], lhsT=ident[0:104, 0:104],
                    rhs=c104_sb[:, 128 * p : 128 * p + 128],
                    start=True, stop=False, skip_group_check=True,
                )

            # ---- F path: copy row 0 of sub-tile 0 to a contiguous tile
            # (matmul lhsT needs one flat free dim) before folds clobber it
            kt0 = kts[0]
            F_sb = small.tile([128, NPAIR, DIM], FP, tag="F")
            nc.vector.tensor_copy(F_sb[:], kt0[:, :, 0, :])
            nc.tensor.matmul(
                PT_ps[:], lhsT=F_sb[:], rhs=idents,
                start=True, stop=False, skip_group_check=True,
            )
            nc.tensor.matmul(
                FT_ps[:], lhsT=F_sb[:], rhs=ident, start=True, stop=True
            )
            FT_sb = small.tile([128, 128], FP, tag="FTs")
            nc.scalar.copy(FT_sb[:], FT_ps[:])
            for p in range(NPAIR):
                prow = slice(64 * p, 64 * p + 64)
                nc.tensor.matmul(
                    SKQs[p][:],
                    lhsT=c64_sb[prow, 208 + 104 * p : 208 + 104 * p + 104],
                    rhs=FT_sb[prow, :],
                    start=False, stop=False, skip_group_check=True,
                )

            # ---- per-sub-tile: batched fold chain then PT accumulation.
            # Power-of-2 tiles stop folding at 4 rows: the last DVE level
            # writes a row-major par2 tile (contiguous (pair, d) rows) and the
            # PE absorbs the final add as two accumulating matmuls.
            PAR2 = {s for s, rs in enumerate(ROWS[:-1]) if rs in (16, 32)}
            for s, rs in enumerate(ROWS):
                kt = kts[s]
                # level 1 folds fp32 -> fp16 into a scratch tile; the rest
                # of the chain runs at DVE 2x (16-bit) rate
                h1 = rs // 2
                k16 = kpool.tile(
                    [128, NPAIR, h1, DIM], F16, tag=f"k16_{s}", name=f"k16_{s}"
                )
                nc.vector.tensor_add(
                    k16[:], kt[:, :, 0:h1, :], kt[:, :, h1 : 2 * h1, :]
                )
                if s in PAR2:
                    ops = []
                    n = h1
                    while n > 4:
                        h = n // 2
                        ops.append((0, h, h, 2 * h))
                        n = h
                    for dlo, dhi, slo, shi in ops:
                        nc.vector.tensor_add(
                            k16[:, :, dlo:dhi, :],
                            k16[:, :, dlo:dhi, :],
                            k16[:, :, slo:shi, :],
                        )
                    par2 = kpool.tile(
                        [128, 2, NPAIR, DIM], F16, tag=f"par{s}", name=f"par{s}"
                    )
                    nc.vector.tensor_add(
                        par2.rearrange("p r q d -> p q r d"),
                        k16[:, :, 0:2, :],
                        k16[:, :, 2:4, :],
                    )
                    for r in range(2):
                        nc.tensor.matmul(
                            PT_ps[:], lhsT=par2[:, r], rhs=lmat_s,
                            start=False,
                            stop=s == nsub - 1 and r == 1,
                            skip_group_check=True,
                        )
                else:
                    ops, (fa, fb) = _chain(h1)
                    for dlo, dhi, slo, shi in ops:
                        nc.vector.tensor_add(
                            k16[:, :, dlo:dhi, :],
                            k16[:, :, dlo:dhi, :],
                            k16[:, :, slo:shi, :],
                        )
                    par = kpool.tile(
                        [128, NPAIR, DIM], F16, tag=f"par{s}", name=f"par{s}"
                    )
                    nc.vector.tensor_add(par[:], k16[:, :, fa, :], k16[:, :, fb, :])
                    nc.tensor.matmul(
                        PT_ps[:], lhsT=par[:], rhs=lmat_s,
                        start=False, stop=s == nsub - 1, skip_group_check=True,
                    )
                if s == 0:
                    # warm-up burst: sustained PE activity flips the HAM clock
                    # gate to 8/8; later matmuls arrive <3.4us apart and hold it
                    for _ in range(32):
                        nc.tensor.matmul(
                            dummy[:], lhsT=cb_sb[:], rhs=cb_sb[:],
                            start=True, stop=True, skip_group_check=True,
                        )

            # ---- tail: PT -> SBUF -> sort projections -> R -> softmax -> out
            PT_sb = small.tile([128, 128], FP, tag="PTs")
            nc.vector.tensor_copy(PT_sb[:], PT_ps[:])
            for p in range(NPAIR):
                prow = slice(64 * p, 64 * p + 64)
                nc.tensor.matmul(
                    SKQs[p][:],
                    lhsT=c64_sb[prow, 104 * p : 104 * p + 104],
                    rhs=PT_sb[prow, :],
                    start=False, stop=True, skip_group_check=True,
                )
            # SQ on scalar, RK on vector: the two copies of each pair overlap
            SQs = []
            RKs = []
            for p in range(NPAIR):
                sq_sb = small.tile([40, 128], FP, tag=f"SQ{p}", name=f"SQ{p}")
                nc.scalar.copy(sq_sb[:], SKQs[p][0:40, :])
                rk_sb = small.tile([40, 128], FP, tag=f"RK{p}", name=f"RK{p}")
                nc.vector.tensor_copy(rk_sb[:], SKQs[p][64:104, :])
                SQs.append(sq_sb)
                RKs.append(rk_sb)
            for p in range(NPAIR):
                nc.tensor.matmul(
                    R_ps[0:64, p, 1:],
                    lhsT=SQs[p][0:8, 0:64],
                    rhs=RKs[p][0:8, 0:64],
                    start=False, stop=False, skip_group_check=True,
                )
                nc.tensor.matmul(
                    R_ps[64:128, p, 1:],
                    lhsT=SQs[p][32:40, 64:128],
                    rhs=RKs[p][32:40, 64:128],
                    start=False, stop=p == NPAIR - 1, skip_group_check=True,
                )

            # masked softmax over 65 logits (zero-logit col 0 is explicit in
            # PSUM from the seed); pair-staggered so pair 0's output DMA is
            # in flight while pair 1 is still in softmax
            mx = small.tile([128, NPAIR], FP, tag="mx")
            nc.vector.reduce_max(mx[:], R_ps[:], axis=X)
            negm = small.tile([128, NPAIR], FP, tag="negm")
            nc.vector.tensor_scalar(
                negm[:], mx[:], 0.0, -1.0,
                op0=mybir.AluOpType.max, op1=MULT,
            )
            e_sb = small.tile([128, NPAIR, BUCKETS + 1], FP, tag="e")
            s1 = small.tile([128, NPAIR], FP, tag="s1")
            rin = small.tile([128, NPAIR], FP, tag="rin")
            outt = small.tile([128, NPAIR, BUCKETS + 1], FP, tag="outt")
            for p in range(NPAIR):
                nc.scalar.activation(
                    e_sb[:, p, :], R_ps[:, p, :], Exp,
                    bias=negm[:, p : p + 1], scale=1.0,
                )
                nc.vector.reduce_sum(s1[:, p : p + 1], e_sb[:, p, :], axis=X)
                nc.vector.reciprocal(rin[:, p : p + 1], s1[:, p : p + 1])
                # outt = (e * 1/den) * tril-mask, fused
                nc.vector.scalar_tensor_tensor(
                    outt[:, p, :],
                    e_sb[:, p, :],
                    rin[:, p : p + 1],
                    mmask,
                    op0=MULT,
                    op1=MULT,
                )
                dst = rout[:, :, p, :].rearrange("b i c -> (b i) c")
                if p == 0:
                    nc.scalar.dma_start(dst, outt[:, p, :])
                else:
                    nc.sync.dma_start(dst, outt[:, p, :])

    nc.compile()
    return nc


def _get_program(enable_asserts=False):
    key = enable_asserts
    if key not in _PROG_CACHE:
        _PROG_CACHE[key] = _build_program(enable_asserts=enable_asserts)
    return _PROG_CACHE[key]


def _host_constants(core, q_pos_emb, k_pos_emb, Wsq, Wsk):
    """Tiny per-core packed constant tensors."""
    f32 = np.float32
    j = np.arange(64, dtype=np.float64)
    s = (1.0 / (CHUNK * j + 1.0)).astype(f32)  # per-bucket cumavg scale

    tri = np.triu(np.ones((64, 64), f32), k=1)  # [c, j] = 1 iff c < j
    tri_s = tri * s[None, :]
    lmat_s = np.zeros((128, 128), f32)
    lmat_s[0:64, 0:64] = tri_s
    lmat_s[64:128, 64:128] = tri_s
    idents = np.zeros((128, 128), f32)
    idents[np.arange(128), np.arange(128)] = np.concatenate([s, s])
    ident = np.eye(128, dtype=f32)

    q = np.arange(64)[:, None]
    jc = np.arange(65)[None, :]
    am = np.where(jc > q, NEG, 0.0).astype(f32)   # softmax additive mask, col0 free
    mm = (jc < q).astype(f32)                     # output tril(-1) mask incl col0
    amask = np.concatenate([am, am], axis=0)      # (128, 65) both b blocks
    mmask = np.concatenate([mm, mm], axis=0)

    c128 = np.concatenate([lmat_s, idents, ident, amask, mmask], axis=1)

    import ml_dtypes

    cb16 = np.eye(128, dtype=ml_dtypes.bfloat16)

    wq_pt = np.zeros((2, 64, 104), f32)   # [pair][d][sq 0:40 | sk 64:104]
    wq_ft = np.zeros((2, 64, 104), f32)
    cblk = np.zeros((2, 104, 128), f32)   # [pair][skq-row][(b, j)]
    for p in range(NPAIR):
        for b in range(2):
            bh = core * BHC + 2 * p + b
            h = bh % HEADS
            r0 = 32 * b
            wq_pt[p, :, r0 : r0 + 8] = Wsq[0, h, 0:64, :]
            wq_pt[p, :, 64 + r0 : 64 + r0 + 8] = Wsk[0, h, 0:64, :]
            wq_ft[p, :, r0 : r0 + 8] = Wsq[0, h, 64:128, :]
            wq_ft[p, :, 64 + r0 : 64 + r0 + 8] = Wsk[0, h, 64:128, :]
            cq = q_pos_emb[0, h] @ Wsq[0, h, 128:192, :]  # (64, 8)
            ck = k_pos_emb[0, h] @ Wsk[0, h, 128:192, :]
            cblk[p, r0 : r0 + 8, 64 * b : 64 * b + 64] = cq.T
            cblk[p, 64 + r0 : 64 + r0 + 8, 64 * b : 64 * b + 64] = ck.T

    c64 = np.concatenate([wq_pt[0], wq_pt[1], wq_ft[0], wq_ft[1]], axis=1)
    c64 = np.concatenate([c64, c64], axis=0)  # duplicate into both halves
    c104 = np.concatenate([cblk[0], cblk[1]], axis=1)
    c104p = np.zeros((128, 256), f32)
    c104p[0:104] = c104
    cM = np.concatenate([c128, c64, c104p], axis=1)
    c16 = lmat_s.astype(np.float16)
    return {"cM": cM, "cb": cb16, "c16": c16}


def _run(k, q_pos_emb, k_pos_emb, Wsq, Wsk, trace=False):
    nc = _get_program()
    in_maps = []
    for core in range(NCORES):
        cm = _host_constants(core, q_pos_emb, k_pos_emb, Wsq, Wsk)
        cm["kin"] = np.ascontiguousarray(k[core * BHC : (core + 1) * BHC])
        in_maps.append(cm)
    res = bass_utils.run_bass_kernel_spmd(
        nc,
        in_maps,
        core_ids=list(range(NCORES)),
        trace=trace,
        **(TRACE_KWARGS if trace else {}),
    )
    global LAST_RESULTS
    LAST_RESULTS = res
    out = np.empty((BH, BUCKETS, BUCKETS + 1), np.float32)
    for core, r in enumerate(res.results):
        ro = r["rout"]  # (2, 64, 2, 65) = (b, i, pair, col)
        for p in range(NPAIR):
            for b in range(2):
                out[core * BHC + 2 * p + b] = ro[b, :, p, :]
    return out, res


def kernel(**inputs):
    k = np.asarray(inputs["k"], np.float32)
    q_pos_emb = np.asarray(inputs["q_pos_emb"], np.float32)
    k_pos_emb = np.asarray(inputs["k_pos_emb"], np.float32)
    Wsq = np.asarray(inputs["Wsq"], np.float32)
    Wsk = np.asarray(inputs["Wsk"], np.float32)
    out, _ = _run(k, q_pos_emb, k_pos_emb, Wsq, Wsk, trace=TRACE)
    return out
